# revision 47
# baseline (speedup 1.0000x reference)
"""CrossAttention kernel for 8 TRN2 NeuronCores.

Sharding: 8 cores = 4 batches x 2 query-halves (zero communication).
Each core computes all 16 heads for its 1024 queries:
  q^T = Wq^T x^T, k^T = Wk^T ctx^T, v = ctx Wv          (bf16 matmuls)
  scores^T[kpos, q] = k^T.T q^T / 8                      (K=64 per head,
      even/odd head pairs issued adjacently -> PE row-group concurrency)
  exp on ScalarE straight from PSUM (no max subtraction; scores ~ N(0,1))
  attn_out^T[d, q] + denominators via ones-augmented V (M=65 matmuls)
  batched reciprocal of all 16 head denominators, per-head GPSIMD
  partition-broadcast, out-proj with bias as a K=1 accumulating matmul.
"""

import sys

for _p in ("/opt/trn_rl_repo", "/root/.axon_site/_ro/trn_rl_repo"):
    if _p not in sys.path:
        sys.path.append(_p)

import numpy as np

import concourse.bass as bass
import concourse.tile as tile
from concourse import bacc, mybir
from concourse.bass_utils import run_bass_kernel_spmd

F32 = mybir.dt.float32
BF16 = mybir.dt.bfloat16
EXP = mybir.ActivationFunctionType.Exp
MULT = mybir.AluOpType.mult

P = 128
B, NQ_FULL, DQ = 4, 2048, 1024
NK, DC = 1024, 768
H, DH = 16, 64
INNER = H * DH  # 1024
NT = 1024  # local queries per core
N_CORES = 8

KQ = DQ // P      # 8
KC = DC // P      # 6
KI = INNER // P   # 8
TB = NT // P      # 8
KB = NK // P      # 8
SCALE = 1.0 / np.sqrt(DH)


def build(dbg=False):
    nc = bacc.Bacc("TRN2", target_bir_lowering=False, debug=False,
                   enable_asserts=False, num_devices=N_CORES)

    x_d = nc.dram_tensor("x", [NT, DQ], BF16, kind="ExternalInput")
    ctx_d = nc.dram_tensor("ctx", [NK, DC], BF16, kind="ExternalInput")
    wq_d = nc.dram_tensor("wq", [DQ, INNER], BF16, kind="ExternalInput")
    wk_d = nc.dram_tensor("wk", [DC, INNER], BF16, kind="ExternalInput")
    wv_d = nc.dram_tensor("wv", [DC, INNER], BF16, kind="ExternalInput")
    wo_d = nc.dram_tensor("wo", [INNER, DQ], BF16, kind="ExternalInput")
    bo_d = nc.dram_tensor("bo", [DQ], BF16, kind="ExternalInput")
    out_d = nc.dram_tensor("out", [NT, DQ], F32, kind="ExternalOutput")
    if dbg:
        dqT = nc.dram_tensor("dqT", [P, KI, NT], F32, kind="ExternalOutput")
        dkT = nc.dram_tensor("dkT", [P, KI, NK], F32, kind="ExternalOutput")
        dvA = nc.dram_tensor("dvA", [P, KB, H, DH + 1], F32, kind="ExternalOutput")
        dav = nc.dram_tensor("dav", [P, H // 2, NT], F32, kind="ExternalOutput")
        dsums = nc.dram_tensor("dsums", [H, NT], F32, kind="ExternalOutput")
        drec = nc.dram_tensor("drec", [H, NT], F32, kind="ExternalOutput")
        dattnT = nc.dram_tensor("dattnT", [P, KI, NT], F32, kind="ExternalOutput")

    dmae = [nc.sync, nc.scalar]  # HWDGE dispatchers, round-robined

    with tile.TileContext(nc) as tc:
        with (
            tc.tile_pool(name="persist", bufs=1) as persist,
            tc.tile_pool(name="psA", bufs=3, space="PSUM") as psA,
            tc.tile_pool(name="psV", bufs=2, space="PSUM") as psV,
        ):
            qT = persist.tile([P, KI, NT], BF16)     # [inner, q]
            kT = persist.tile([P, KI, NK], BF16)     # [inner, kpos]
            vA = persist.tile([P, KB, H, DH + 1], BF16)  # [kpos, (head, d|1)]
            attnT = persist.tile([P, KI, NT], BF16)  # [inner, q]
            wo_b = persist.tile([P, KI, DQ], BF16)
            bo_b = persist.tile([1, DQ], BF16)
            ones_b = persist.tile([1, P], BF16)

            dmai = 0

            def dma(out, in_):
                nonlocal dmai
                dmae[dmai % 2].dma_start(out, in_)
                dmai += 1

            def dmaT(out, in_):
                nc.sync.dma_start_transpose(out, in_)

            with tc.tile_pool(name="stage", bufs=1) as stage, \
                 tc.tile_pool(name="stage2", bufs=3) as stage2:
                xT = stage.tile([P, KQ, NT], BF16, tag="xT")
                cT = stage.tile([P, KC, NK], BF16, tag="cT")
                wq_b = stage.tile([P, KQ, INNER], BF16, tag="wq")
                wk_b = stage.tile([P, KC, INNER], BF16, tag="wk")
                wv_b = stage.tile([P, KC, INNER], BF16, tag="wv")

                nc.gpsimd.memset(ones_b[:], 1.0)
                nc.gpsimd.memset(vA[:, :, :, DH:DH + 1], 1.0)

                # wq: load bf16 straight into place (per-subtile chunks)
                wq4 = wq_d.ap().rearrange("(o p) m -> p o m", p=P)
                for ko in range(KQ):
                    dma(wq_b[:, ko], wq4[:, ko])
                # x: load natural bf16 (scalar queue), then batched transposes
                # (sync queue stays in a single XBAR mode per batch)
                x4 = x_d.ap().rearrange("(t p) d -> p t d", p=P)
                xbs = []
                for t in range(TB):
                    xb = stage2.tile([P, DQ], BF16, tag="castb", name=f"xb{t}")
                    nc.scalar.dma_start(xb[:], x4[:, t])
                    xbs.append(xb)
                for t in range(TB):
                    dmaT(xT[:, :, t * P:(t + 1) * P], xbs[t][:])

                # ---- q projection ----
                for ko in range(KI):
                    for n0 in range(0, NT, 512):
                        ps = psA.tile([P, 1024], F32, tag="big")
                        for kc in range(KQ):
                            nc.tensor.matmul(
                                ps[:, 0:512],
                                wq_b[:, kc, ko * P:(ko + 1) * P],
                                xT[:, kc, n0:n0 + 512],
                                start=(kc == 0), stop=(kc == KQ - 1))
                        nc.vector.tensor_copy(qT[:, ko, n0:n0 + 512],
                                              ps[:, 0:512])

                # ctx / wk / wv / wo / bo loads (bf16, direct)
                c4 = ctx_d.ap().rearrange("(t p) d -> p t d", p=P)
                cbs = []
                for t in range(KB):
                    cb = stage2.tile([P, DQ], BF16, tag="castb", name=f"cb{t}")
                    nc.scalar.dma_start(cb[:, :DC], c4[:, t])
                    cbs.append(cb)
                for t in range(KB):
                    dmaT(cT[:, :, t * P:(t + 1) * P], cbs[t][:, :DC])
                wk4 = wk_d.ap().rearrange("(o p) m -> p o m", p=P)
                wv4 = wv_d.ap().rearrange("(o p) m -> p o m", p=P)
                for ko in range(KC):
                    dma(wk_b[:, ko], wk4[:, ko])
                    dma(wv_b[:, ko], wv4[:, ko])
                wo4 = wo_d.ap().rearrange("(o p) m -> p o m", p=P)
                for ko in range(KI):
                    dma(wo_b[:, ko], wo4[:, ko])
                dma(bo_b[:], bo_d.ap()[None, :])

                # ---- k / v projections ----
                for ko in range(KI):
                    for n0 in range(0, NK, 512):
                        ps = psA.tile([P, 1024], F32, tag="big")
                        for kc in range(KC):
                            nc.tensor.matmul(
                                ps[:, 0:512],
                                wk_b[:, kc, ko * P:(ko + 1) * P],
                                cT[:, kc, n0:n0 + 512],
                                start=(kc == 0), stop=(kc == KC - 1))
                        nc.vector.tensor_copy(kT[:, ko, n0:n0 + 512],
                                              ps[:, 0:512])
                for mt in range(KB):
                    for n0 in range(0, INNER, 512):
                        ps = psA.tile([P, 1024], F32, tag="big")
                        for kc in range(KC):
                            nc.tensor.matmul(
                                ps[:, 0:512],
                                cT[:, kc, mt * P:(mt + 1) * P],
                                wv_b[:, kc, n0:n0 + 512],
                                start=(kc == 0), stop=(kc == KC - 1))
                        h0 = n0 // DH
                        nc.vector.tensor_copy(
                            vA[:, mt, h0:h0 + 8, 0:DH],
                            ps[:, 0:512].rearrange("p (h d) -> p h d", d=DH))

            # ---------------- attention, head pairs ----------------
            expp = tc.alloc_tile_pool(name="expp", bufs=16)
            avp = tc.alloc_tile_pool(name="avp", bufs=1)
            dramp = tc.alloc_tile_pool(name="dramp", bufs=1, space="DRAM")
            sums_dram = dramp.tile([H, NT], F32, name="sums_dram")
            av_sb = avp.tile([P, H // 2, NT], BF16, tag="avsb")  # [2*64d, hpair, q]
            def attn_head_pair(hp):
                h0, h1 = 2 * hp, 2 * hp + 1
                h2 = hp
                psvs = {h: [psV.tile([DH + 1, 512], F32, tag="av",
                                     name=f"psv{h}_{n}") for n in (0, 1)]
                        for h in (h0, h1)}
                ets_all = {h0: [], h1: []}
                for kb in range(KB):
                    pss = {}
                    for h in (h0, h1):
                        base = (h % 2) * DH
                        ps = psA.tile([P, 1024], F32, tag="big")
                        pss[h] = ps
                        for n0 in (0, 512):
                            nc.tensor.matmul(
                                ps[:, n0:n0 + 512],
                                kT[base:base + DH, h2, kb * P:(kb + 1) * P],
                                qT[base:base + DH, h2, n0:n0 + 512],
                                start=True, stop=True)
                    for h in (h0, h1):
                        et = expp.tile([P, NT], BF16, tag="exp")
                        nc.scalar.activation(et[:], pss[h][:], EXP,
                                             scale=float(SCALE))
                        ets_all[h].append(et)
                    for h in (h0, h1):
                        for ni, n0 in enumerate((0, 512)):
                            nc.tensor.matmul(
                                psvs[h][ni][:],
                                vA[:, kb, h, :],
                                ets_all[h][kb][:, n0:n0 + 512],
                                start=(kb == 0), stop=(kb == KB - 1))
                for i, h in enumerate((h0, h1)):
                    for ni, n0 in enumerate((0, 512)):
                        srow = expp.tile([1, 512], F32, tag="srow",
                                         name=f"srow{h}_{ni}")
                        nc.vector.tensor_copy(srow[:], psvs[h][ni][DH:DH + 1, :])
                        dma(sums_dram[h:h + 1, n0:n0 + 512], srow[:])
                        nc.vector.tensor_copy(
                            av_sb[i * DH:(i + 1) * DH, hp, n0:n0 + 512],
                            psvs[h][ni][0:DH, :])

            def normalize_batch(bi):
                nh = 32  # rows in the [128,128] reshaped view per 4-head batch
                sums8 = avp.tile([nh, P], F32, tag=f"sums{bi}",
                                 name=f"sums_b{bi}")
                dma(sums8[:], sums_dram[:]
                    .rearrange("h (a b) -> (h a) b", b=P)[bi * nh:(bi + 1) * nh])
                rec8 = avp.tile([nh, P], F32, tag=f"rec{bi}",
                                name=f"rec_b{bi}")
                nc.vector.reciprocal(rec8[:], sums8[:])
                dma(rec_dram[:]
                    .rearrange("h (a b) -> (h a) b", b=P)[bi * nh:(bi + 1) * nh],
                    rec8[:])
                for hp in range(bi * 2, (bi + 1) * 2):
                    rb = recp.tile([P, NT], F32, tag="rb")
                    for i in (0, 1):
                        src = rec_dram[2 * hp + i:2 * hp + i + 1, :]
                        bsrc = bass.AP(tensor=src.tensor, offset=src.offset,
                                       ap=[[0, DH]] + list(src.ap[1:]))
                        dma(rb[i * DH:(i + 1) * DH, :], bsrc)
                    nc.vector.tensor_tensor(attnT[:, hp, :],
                                            av_sb[:, hp, :],
                                            rb[:], MULT)

            recp = tc.alloc_tile_pool(name="recp", bufs=4)
            rec_dram = dramp.tile([H, NT], F32, name="rec_dram")
            for hp in range(H // 2):
                attn_head_pair(hp)
                if hp in (1, 3, 5):
                    normalize_batch(hp // 2)
            normalize_batch(3)

            def dbg_dump(nc, tc, dma):
                dbgp = tc.alloc_tile_pool(name="dbgp", bufs=2)
                for name, t_sb, t_d in ((
                    "qT", qT, dqT), ("kT", kT, dkT), ("attnT", attnT, dattnT)):
                    for ko in range(KI):
                        f = dbgp.tile([P, NT], F32, tag="dbgf", name=f"dbg_{name}{ko}")
                        nc.vector.tensor_copy(f[:], t_sb[:, ko])
                        dma(t_d.ap().rearrange("p k n -> p k n")[:, ko], f[:])
                for kb in range(KB):
                    f = dbgp.tile([P, H * (DH + 1)], F32, tag="dbgf", name=f"dbg_v{kb}")
                    nc.vector.tensor_copy(f[:].rearrange("p (h d) -> p h d", d=DH+1), vA[:, kb])
                    dma(dvA.ap()[:, kb], f[:].rearrange("p (h d) -> p h d", d=DH+1))
                for hp2 in range(H // 2):
                    f = dbgp.tile([P, NT], F32, tag="dbgf", name=f"dbg_av{hp2}")
                    nc.vector.tensor_copy(f[:], av_sb2[:, hp2])
                    dma(dav.ap()[:, hp2], f[:])
                dma(dsums.ap(), sums16[:])
                dma(drec.ap(), rec16[:])
                dbgp.release()


            av_sb2 = av_sb
            if dbg:
                dbg_dump(nc, tc, dma)
            recp.release()
            avp.release()
            expp.release()
            dramp.release()

            # ---------------- out projection + bias ----------------
            outp = tc.alloc_tile_pool(name="outp", bufs=2)
            out3 = out_d.ap().rearrange("(t p) d -> p t d", p=P)
            for mt in range(TB):
                ps = psA.tile([P, 1024], F32, tag="big")
                for n0 in (0, 512):
                    for kc in range(KI):
                        nc.tensor.matmul(
                            ps[:, n0:n0 + 512],
                            attnT[:, kc, mt * P:(mt + 1) * P],
                            wo_b[:, kc, n0:n0 + 512],
                            start=(kc == 0), stop=False)
                    nc.tensor.matmul(
                        ps[:, n0:n0 + 512],
                        ones_b[0:1, :],
                        bo_b[0:1, n0:n0 + 512],
                        start=False, stop=True)
                ot = outp.tile([P, DQ], F32, tag="out")
                nc.vector.tensor_copy(ot[:], ps[:])
                dma(out3[:, mt], ot[:])
            outp.release()

    nc.compile()
    return nc


_NC_CACHE = None


def _make_in_maps(inputs):
    import ml_dtypes
    bf = ml_dtypes.bfloat16
    x = np.ascontiguousarray(
        np.asarray(inputs["x"], dtype=np.float32).astype(bf))
    context = np.ascontiguousarray(
        np.asarray(inputs["context"], dtype=np.float32).astype(bf))
    shared = {
        "wq": np.ascontiguousarray(np.asarray(inputs["Wq"], np.float32).astype(bf)),
        "wk": np.ascontiguousarray(np.asarray(inputs["Wk"], np.float32).astype(bf)),
        "wv": np.ascontiguousarray(np.asarray(inputs["Wv"], np.float32).astype(bf)),
        "wo": np.ascontiguousarray(np.asarray(inputs["Wo"], np.float32).astype(bf)),
        "bo": np.ascontiguousarray(np.asarray(inputs["bo"], np.float32).astype(bf)),
    }
    in_maps = []
    for c in range(N_CORES):
        b, s = divmod(c, 2)
        in_maps.append({
            "x": np.ascontiguousarray(x[b, s * NT:(s + 1) * NT, :]),
            "ctx": np.ascontiguousarray(context[b]),
            **shared,
        })
    return in_maps


def kernel(x, context, Wq, Wk, Wv, Wo, bo):
    global _NC_CACHE
    if _NC_CACHE is None:
        _NC_CACHE = build()
    nc = _NC_CACHE

    in_maps = _make_in_maps(dict(x=x, context=context, Wq=Wq, Wk=Wk, Wv=Wv,
                                 Wo=Wo, bo=bo))
    res = run_bass_kernel_spmd(nc, in_maps, core_ids=list(range(N_CORES)))
    out = np.empty((B, NQ_FULL, DQ), dtype=np.float32)
    for c in range(N_CORES):
        b, s = divmod(c, 2)
        out[b, s * NT:(s + 1) * NT, :] = res.results[c]["out"]
    return out


# revision 48
# speedup vs baseline: 1.1785x; 1.1785x over previous
"""CrossAttention kernel for 8 TRN2 NeuronCores.

Sharding: 8 cores = 4 batches x 2 query-halves (zero communication).
Each core computes all 16 heads for its 1024 queries:
  q^T = Wq^T x^T, k^T = Wk^T ctx^T, v = ctx Wv          (bf16 matmuls)
  scores^T[kpos, q] = k^T.T q^T / 8                      (K=64 per head,
      even/odd head pairs issued adjacently -> PE row-group concurrency)
  exp on ScalarE straight from PSUM (no max subtraction; scores ~ N(0,1))
  attn_out^T[d, q] + denominators via ones-augmented V (M=65 matmuls)
  batched reciprocal of all 16 head denominators, per-head GPSIMD
  partition-broadcast, out-proj with bias as a K=1 accumulating matmul.
"""

import sys

for _p in ("/opt/trn_rl_repo", "/root/.axon_site/_ro/trn_rl_repo"):
    if _p not in sys.path:
        sys.path.append(_p)

import numpy as np

import concourse.bass as bass
import concourse.tile as tile
from concourse import bacc, mybir
from concourse.bass_utils import run_bass_kernel_spmd

F32 = mybir.dt.float32
BF16 = mybir.dt.bfloat16
EXP = mybir.ActivationFunctionType.Exp
MULT = mybir.AluOpType.mult

P = 128
B, NQ_FULL, DQ = 4, 2048, 1024
NK, DC = 1024, 768
H, DH = 16, 64
INNER = H * DH  # 1024
NT = 1024  # local queries per core
N_CORES = 8

KQ = DQ // P      # 8
KC = DC // P      # 6
KI = INNER // P   # 8
TB = NT // P      # 8
KB = NK // P      # 8
SCALE = 1.0 / np.sqrt(DH)


def build(dbg=False):
    nc = bacc.Bacc("TRN2", target_bir_lowering=False, debug=False,
                   enable_asserts=False, num_devices=N_CORES)

    x_d = nc.dram_tensor("x", [NT, DQ], BF16, kind="ExternalInput")
    ctx_d = nc.dram_tensor("ctx", [NK, DC], BF16, kind="ExternalInput")
    wq_d = nc.dram_tensor("wq", [DQ, INNER], BF16, kind="ExternalInput")
    wk_d = nc.dram_tensor("wk", [DC, INNER], BF16, kind="ExternalInput")
    wv_d = nc.dram_tensor("wv", [DC, INNER], BF16, kind="ExternalInput")
    wo_d = nc.dram_tensor("wo", [INNER, DQ], BF16, kind="ExternalInput")
    bo_d = nc.dram_tensor("bo", [DQ], BF16, kind="ExternalInput")
    out_d = nc.dram_tensor("out", [NT, DQ], F32, kind="ExternalOutput")
    if dbg:
        dqT = nc.dram_tensor("dqT", [P, KI, NT], F32, kind="ExternalOutput")
        dkT = nc.dram_tensor("dkT", [P, KI, NK], F32, kind="ExternalOutput")
        dvA = nc.dram_tensor("dvA", [P, KB, H, DH + 1], F32, kind="ExternalOutput")
        dav = nc.dram_tensor("dav", [P, H // 2, NT], F32, kind="ExternalOutput")
        dsums = nc.dram_tensor("dsums", [H, NT], F32, kind="ExternalOutput")
        drec = nc.dram_tensor("drec", [H, NT], F32, kind="ExternalOutput")
        dattnT = nc.dram_tensor("dattnT", [P, KI, NT], F32, kind="ExternalOutput")

    dmae = [nc.sync, nc.scalar]  # HWDGE dispatchers, round-robined

    with tile.TileContext(nc) as tc:
        with (
            tc.tile_pool(name="persist", bufs=1) as persist,
            tc.tile_pool(name="psA", bufs=3, space="PSUM") as psA,
            tc.tile_pool(name="psV", bufs=2, space="PSUM") as psV,
        ):
            qT = persist.tile([P, KI, NT], BF16)     # [inner, q]
            kT = persist.tile([P, KI, NK], BF16)     # [inner, kpos]
            vA = persist.tile([P, KB, H, DH + 1], BF16)  # [kpos, (head, d|1)]
            attnT = persist.tile([P, KI, NT], BF16)  # [inner, q]
            wo_b = persist.tile([P, KI, DQ], BF16)
            bo_b = persist.tile([1, DQ], BF16)
            ones_b = persist.tile([1, P], BF16)

            dmai = 0

            def dma(out, in_):
                nonlocal dmai
                dmae[dmai % 2].dma_start(out, in_)
                dmai += 1

            def dmaT(out, in_):
                nc.sync.dma_start_transpose(out, in_)

            with tc.tile_pool(name="stage", bufs=1) as stage, \
                 tc.tile_pool(name="stage2", bufs=3) as stage2:
                xT = stage.tile([P, KQ, NT], BF16, tag="xT")
                cT = stage.tile([P, KC, NK], BF16, tag="cT")
                wq_b = stage.tile([P, KQ, INNER], BF16, tag="wq")
                wk_b = stage.tile([P, KC, INNER], BF16, tag="wk")
                wv_b = stage.tile([P, KC, INNER], BF16, tag="wv")

                nc.gpsimd.memset(ones_b[:], 1.0)
                nc.gpsimd.memset(vA[:, :, :, DH:DH + 1], 1.0)

                # wq: load bf16 straight into place (per-subtile chunks)
                wq4 = wq_d.ap().rearrange("(o p) m -> p o m", p=P)
                for ko in range(KQ):
                    dma(wq_b[:, ko], wq4[:, ko])
                # x: load natural bf16, big transpose
                x4 = x_d.ap().rearrange("(t p) d -> p t d", p=P)
                for t in range(TB):
                    xb = stage2.tile([P, DQ], BF16, tag="castb")
                    dma(xb[:], x4[:, t])
                    dmaT(xT[:, :, t * P:(t + 1) * P], xb[:])

                # ---- q projection ----
                for ko in range(KI):
                    for n0 in range(0, NT, 512):
                        ps = psA.tile([P, 1024], F32, tag="big")
                        for kc in range(KQ):
                            nc.tensor.matmul(
                                ps[:, 0:512],
                                wq_b[:, kc, ko * P:(ko + 1) * P],
                                xT[:, kc, n0:n0 + 512],
                                start=(kc == 0), stop=(kc == KQ - 1))
                        nc.vector.tensor_copy(qT[:, ko, n0:n0 + 512],
                                              ps[:, 0:512])

                # ctx / wk / wv / wo / bo loads (bf16, direct)
                c4 = ctx_d.ap().rearrange("(t p) d -> p t d", p=P)
                for t in range(KB):
                    cb = stage2.tile([P, DQ], BF16, tag="castb")
                    dma(cb[:, :DC], c4[:, t])
                    dmaT(cT[:, :, t * P:(t + 1) * P], cb[:, :DC])
                wk4 = wk_d.ap().rearrange("(o p) m -> p o m", p=P)
                wv4 = wv_d.ap().rearrange("(o p) m -> p o m", p=P)
                for ko in range(KC):
                    dma(wk_b[:, ko], wk4[:, ko])
                    dma(wv_b[:, ko], wv4[:, ko])
                wo4 = wo_d.ap().rearrange("(o p) m -> p o m", p=P)
                for ko in range(KI):
                    dma(wo_b[:, ko], wo4[:, ko])
                dma(bo_b[:], bo_d.ap()[None, :])

                # ---- k / v projections ----
                for ko in range(KI):
                    for n0 in range(0, NK, 512):
                        ps = psA.tile([P, 1024], F32, tag="big")
                        for kc in range(KC):
                            nc.tensor.matmul(
                                ps[:, 0:512],
                                wk_b[:, kc, ko * P:(ko + 1) * P],
                                cT[:, kc, n0:n0 + 512],
                                start=(kc == 0), stop=(kc == KC - 1))
                        nc.vector.tensor_copy(kT[:, ko, n0:n0 + 512],
                                              ps[:, 0:512])
                for mt in range(KB):
                    for n0 in range(0, INNER, 512):
                        ps = psA.tile([P, 1024], F32, tag="big")
                        for kc in range(KC):
                            nc.tensor.matmul(
                                ps[:, 0:512],
                                cT[:, kc, mt * P:(mt + 1) * P],
                                wv_b[:, kc, n0:n0 + 512],
                                start=(kc == 0), stop=(kc == KC - 1))
                        h0 = n0 // DH
                        nc.vector.tensor_copy(
                            vA[:, mt, h0:h0 + 8, 0:DH],
                            ps[:, 0:512].rearrange("p (h d) -> p h d", d=DH))

            # ---------------- attention, head pairs ----------------
            expp = tc.alloc_tile_pool(name="expp", bufs=16)
            avp = tc.alloc_tile_pool(name="avp", bufs=1)
            dramp = tc.alloc_tile_pool(name="dramp", bufs=1, space="DRAM")
            sums_dram = dramp.tile([H, NT], F32, name="sums_dram")
            av_sb = avp.tile([P, H // 2, NT], BF16, tag="avsb")  # [2*64d, hpair, q]
            def attn_head_pair(hp):
                h0, h1 = 2 * hp, 2 * hp + 1
                h2 = hp
                psvs = {h: [psV.tile([DH + 1, 512], F32, tag="av",
                                     name=f"psv{h}_{n}") for n in (0, 1)]
                        for h in (h0, h1)}
                ets_all = {h0: [], h1: []}
                for kb in range(KB):
                    pss = {}
                    for h in (h0, h1):
                        base = (h % 2) * DH
                        ps = psA.tile([P, 1024], F32, tag="big")
                        pss[h] = ps
                        for n0 in (0, 512):
                            nc.tensor.matmul(
                                ps[:, n0:n0 + 512],
                                kT[base:base + DH, h2, kb * P:(kb + 1) * P],
                                qT[base:base + DH, h2, n0:n0 + 512],
                                start=True, stop=True)
                    for h in (h0, h1):
                        et = expp.tile([P, NT], BF16, tag="exp")
                        nc.scalar.activation(et[:], pss[h][:], EXP,
                                             scale=float(SCALE))
                        ets_all[h].append(et)
                    for h in (h0, h1):
                        for ni, n0 in enumerate((0, 512)):
                            nc.tensor.matmul(
                                psvs[h][ni][:],
                                vA[:, kb, h, :],
                                ets_all[h][kb][:, n0:n0 + 512],
                                start=(kb == 0), stop=(kb == KB - 1))
                for i, h in enumerate((h0, h1)):
                    for ni, n0 in enumerate((0, 512)):
                        srow = expp.tile([1, 512], F32, tag="srow",
                                         name=f"srow{h}_{ni}")
                        nc.vector.tensor_copy(srow[:], psvs[h][ni][DH:DH + 1, :])
                        dma(sums_dram[h:h + 1, n0:n0 + 512], srow[:])
                        nc.vector.tensor_copy(
                            av_sb[i * DH:(i + 1) * DH, hp, n0:n0 + 512],
                            psvs[h][ni][0:DH, :])

            def normalize_batch(bi):
                nh = 32  # rows in the [128,128] reshaped view per 4-head batch
                sums8 = avp.tile([nh, P], F32, tag=f"sums{bi}",
                                 name=f"sums_b{bi}")
                dma(sums8[:], sums_dram[:]
                    .rearrange("h (a b) -> (h a) b", b=P)[bi * nh:(bi + 1) * nh])
                rec8 = avp.tile([nh, P], F32, tag=f"rec{bi}",
                                name=f"rec_b{bi}")
                nc.vector.reciprocal(rec8[:], sums8[:])
                dma(rec_dram[:]
                    .rearrange("h (a b) -> (h a) b", b=P)[bi * nh:(bi + 1) * nh],
                    rec8[:])
                for hp in range(bi * 2, (bi + 1) * 2):
                    rb = recp.tile([P, NT], F32, tag="rb")
                    for i in (0, 1):
                        src = rec_dram[2 * hp + i:2 * hp + i + 1, :]
                        bsrc = bass.AP(tensor=src.tensor, offset=src.offset,
                                       ap=[[0, DH]] + list(src.ap[1:]))
                        dma(rb[i * DH:(i + 1) * DH, :], bsrc)
                    nc.vector.tensor_tensor(attnT[:, hp, :],
                                            av_sb[:, hp, :],
                                            rb[:], MULT)

            recp = tc.alloc_tile_pool(name="recp", bufs=4)
            rec_dram = dramp.tile([H, NT], F32, name="rec_dram")
            for hp in range(H // 2):
                attn_head_pair(hp)
                if hp in (1, 3, 5):
                    normalize_batch(hp // 2)
            normalize_batch(3)

            def dbg_dump(nc, tc, dma):
                dbgp = tc.alloc_tile_pool(name="dbgp", bufs=2)
                for name, t_sb, t_d in ((
                    "qT", qT, dqT), ("kT", kT, dkT), ("attnT", attnT, dattnT)):
                    for ko in range(KI):
                        f = dbgp.tile([P, NT], F32, tag="dbgf", name=f"dbg_{name}{ko}")
                        nc.vector.tensor_copy(f[:], t_sb[:, ko])
                        dma(t_d.ap().rearrange("p k n -> p k n")[:, ko], f[:])
                for kb in range(KB):
                    f = dbgp.tile([P, H * (DH + 1)], F32, tag="dbgf", name=f"dbg_v{kb}")
                    nc.vector.tensor_copy(f[:].rearrange("p (h d) -> p h d", d=DH+1), vA[:, kb])
                    dma(dvA.ap()[:, kb], f[:].rearrange("p (h d) -> p h d", d=DH+1))
                for hp2 in range(H // 2):
                    f = dbgp.tile([P, NT], F32, tag="dbgf", name=f"dbg_av{hp2}")
                    nc.vector.tensor_copy(f[:], av_sb2[:, hp2])
                    dma(dav.ap()[:, hp2], f[:])
                dma(dsums.ap(), sums16[:])
                dma(drec.ap(), rec16[:])
                dbgp.release()


            av_sb2 = av_sb
            if dbg:
                dbg_dump(nc, tc, dma)
            recp.release()
            avp.release()
            expp.release()
            dramp.release()

            # ---------------- out projection + bias ----------------
            outp = tc.alloc_tile_pool(name="outp", bufs=2)
            out3 = out_d.ap().rearrange("(t p) d -> p t d", p=P)
            for mt in range(TB):
                ps = psA.tile([P, 1024], F32, tag="big")
                for n0 in (0, 512):
                    for kc in range(KI):
                        nc.tensor.matmul(
                            ps[:, n0:n0 + 512],
                            attnT[:, kc, mt * P:(mt + 1) * P],
                            wo_b[:, kc, n0:n0 + 512],
                            start=(kc == 0), stop=False)
                    nc.tensor.matmul(
                        ps[:, n0:n0 + 512],
                        ones_b[0:1, :],
                        bo_b[0:1, n0:n0 + 512],
                        start=False, stop=True)
                ot = outp.tile([P, DQ], F32, tag="out")
                nc.vector.tensor_copy(ot[:], ps[:])
                dma(out3[:, mt], ot[:])
            outp.release()

    nc.compile()
    return nc


_NC_CACHE = None


def _make_in_maps(inputs):
    import ml_dtypes
    bf = ml_dtypes.bfloat16
    x = np.ascontiguousarray(
        np.asarray(inputs["x"], dtype=np.float32).astype(bf))
    context = np.ascontiguousarray(
        np.asarray(inputs["context"], dtype=np.float32).astype(bf))
    shared = {
        "wq": np.ascontiguousarray(np.asarray(inputs["Wq"], np.float32).astype(bf)),
        "wk": np.ascontiguousarray(np.asarray(inputs["Wk"], np.float32).astype(bf)),
        "wv": np.ascontiguousarray(np.asarray(inputs["Wv"], np.float32).astype(bf)),
        "wo": np.ascontiguousarray(np.asarray(inputs["Wo"], np.float32).astype(bf)),
        "bo": np.ascontiguousarray(np.asarray(inputs["bo"], np.float32).astype(bf)),
    }
    in_maps = []
    for c in range(N_CORES):
        b, s = divmod(c, 2)
        in_maps.append({
            "x": np.ascontiguousarray(x[b, s * NT:(s + 1) * NT, :]),
            "ctx": np.ascontiguousarray(context[b]),
            **shared,
        })
    return in_maps


def kernel(x, context, Wq, Wk, Wv, Wo, bo):
    global _NC_CACHE
    if _NC_CACHE is None:
        _NC_CACHE = build()
    nc = _NC_CACHE

    in_maps = _make_in_maps(dict(x=x, context=context, Wq=Wq, Wk=Wk, Wv=Wv,
                                 Wo=Wo, bo=bo))
    res = run_bass_kernel_spmd(nc, in_maps, core_ids=list(range(N_CORES)))
    out = np.empty((B, NQ_FULL, DQ), dtype=np.float32)
    for c in range(N_CORES):
        b, s = divmod(c, 2)
        out[b, s * NT:(s + 1) * NT, :] = res.results[c]["out"]
    return out
